# revision 6
# baseline (speedup 1.0000x reference)
"""Trainium2 Bass kernel for nn_Convattn (dense transformer conv-attention).

Strategy: data-parallel over batch (16 batches -> 8 cores x 2 batches).
Host prep: transpose/cast activations to bf16 channel-major layout, cast
weights to bf16, and build the batch-independent additive attention bias
from the tiny replicated tables (rpb gathers + 2x512x12 pos-MLP), shipped
as exp(bias^T) so the device applies it multiplicatively after the softmax
exp.

Device (per core, one SPMD NEFF):
  QKV:  Q^T/K^T = W^T @ x (channel-major, contraction on partitions),
        V in token-major layout augmented with a per-head ones block so the
        attention-V matmul also produces softmax denominators (partition-dim
        reduction done on the PE).
  Attn: per head, S^T = K_h^T.T @ Q_h^T  (keys on partitions, queries free)
        P^T = exp(S^T) * exp(bias^T)     (ACT exp, DVE/GPSIMD multiply)
        O_aug = V_aug^T @ P^T            (rows 0:64 = O^T, 64:128 = rowsum)
        O^T = O_aug[0:64] * 1/rowsum     (DVE reciprocal + multiply)
  Proj: out = O @ W_p + b_p, DMA to HBM.
"""

import sys

sys.path.insert(0, "/opt/trn_rl_repo")

import numpy as np
import ml_dtypes

# ---- problem constants (hardcoded per spec) ----
TEMP_LEN = 16
TARGET_LEN = 22
DIM = 768
HEADS = 12
HD = 64
B = 16
NT = TEMP_LEN * TEMP_LEN        # 256
NG = TARGET_LEN * TARGET_LEN    # 484
N = NT + NG                     # 740
POS_HID = 512
NCORES = 8
BPC = B // NCORES               # 2 batches per core
P = 128
KT_SIZES = [128, 128, 128, 128, 128, 100]   # 740 keys/tokens -> 6 tiles
NSPLITS = [(0, 512), (512, 740)]            # matmul free-dim splits (<=512 fp32 psum)
D_SPLITS = [(0, 512), (512, 768)]
SCALE = HD ** -0.5

BF16 = ml_dtypes.bfloat16


def _bias_index(L):
    coords = np.stack(np.meshgrid(np.arange(L), np.arange(L), indexing="ij"))
    flat = coords.reshape(2, -1)
    rel = (flat[:, :, None] - flat[:, None, :]).transpose(1, 2, 0).astype(np.int64)
    rel[:, :, 0] += L - 1
    rel[:, :, 1] += L - 1
    rel[:, :, 0] *= 2 * L - 1
    return rel.sum(-1)


def _host_bias(rpb_table, rpb_table_temp, pos_w1, pos_b1, pos_w2, pos_b2,
               target_temp_tab):
    """exp(bias)^T per head: [H, N(keys), N(queries)] float32."""
    temp_idx = _bias_index(TEMP_LEN)
    target_idx = _bias_index(TARGET_LEN)
    temp_rpb = rpb_table_temp[temp_idx.reshape(-1)].reshape(NT, NT, HEADS)
    targ_rpb = rpb_table[target_idx.reshape(-1)].reshape(NG, NG, HEADS)
    tab = np.asarray(target_temp_tab, np.float32).reshape(-1, 2)       # [NG*NT, 2]
    hid = np.maximum(tab @ np.asarray(pos_w1, np.float32)
                     + np.asarray(pos_b1, np.float32), 0.0)
    tt = (hid @ np.asarray(pos_w2, np.float32)
          + np.asarray(pos_b2, np.float32)).reshape(NG, NT, HEADS)     # [NG,NT,H]
    bias = np.empty((HEADS, N, N), np.float32)
    bias[:, :NT, :NT] = temp_rpb.transpose(2, 0, 1)
    bias[:, :NT, NT:] = tt.transpose(2, 1, 0)      # [H, NT, NG]
    bias[:, NT:, :NT] = tt.transpose(2, 0, 1)      # [H, NG, NT]
    bias[:, NT:, NT:] = targ_rpb.transpose(2, 0, 1)
    # transpose to [H, keys, queries] and exponentiate
    return np.exp(bias.transpose(0, 2, 1))


def _build_nc():
    import concourse.bass as bass
    import concourse.tile as tile
    import concourse.mybir as mybir
    from concourse.bacc import Bacc

    f32 = mybir.dt.float32
    bf16 = mybir.dt.bfloat16
    EXP = mybir.ActivationFunctionType.Exp

    nc = Bacc()
    xT = nc.dram_tensor("xT", [BPC, DIM, N], bf16, kind="ExternalInput")
    qT = nc.dram_tensor("qT", [BPC, DIM, N], bf16, kind="ExternalInput")
    wq = nc.dram_tensor("wq", [DIM, DIM], bf16, kind="ExternalInput")
    wk = nc.dram_tensor("wk", [DIM, DIM], bf16, kind="ExternalInput")
    wv = nc.dram_tensor("wv", [DIM, DIM], bf16, kind="ExternalInput")
    wp = nc.dram_tensor("wp", [DIM, DIM], bf16, kind="ExternalInput")
    pbr = nc.dram_tensor("pbr", [P, DIM], f32, kind="ExternalInput")  # proj_b replicated
    eb = nc.dram_tensor("eb", [HEADS, DIM, N], bf16, kind="ExternalInput")  # exp(biasT), key-padded to 768
    out = nc.dram_tensor("out", [BPC, N, DIM], f32, kind="ExternalOutput")

    KS = DIM // P  # 6 contraction subtiles

    with tile.TileContext(nc) as tc:
        with (
            tc.tile_pool(name="qkvp", bufs=1) as qkvp,          # Q^T/K^T/V_aug both batches
            tc.tile_pool(name="psum", bufs=4, space="PSUM") as psum,
            tc.tile_pool(name="work", bufs=3) as work,
        ):
            QT, KTT, VA = [], [], []
            for b in range(BPC):
                QT.append(qkvp.tile([P, KS, N], bf16, tag=f"qt{b}", name=f"qt{b}"))
                KTT.append(qkvp.tile([P, KS, N], bf16, tag=f"kt{b}", name=f"ktt{b}"))
                # V augmented: per head 128 cols = [V_h (64) | ones (64)]
                VA.append(qkvp.tile([P, KS, HEADS * 128], bf16, tag=f"va{b}", name=f"va{b}"))

            # ---------------- QKV phase ----------------
            with tc.tile_pool(name="wpool", bufs=1) as wpool, \
                 tc.tile_pool(name="xin", bufs=3) as xin:
                wq_s = wpool.tile([P, KS, DIM], bf16, tag="wq")
                wk_s = wpool.tile([P, KS, DIM], bf16, tag="wk")
                wv_s = wpool.tile([P, KS, DIM], bf16, tag="wv")
                nc.sync.dma_start(wq_s[:], wq.rearrange("(ko ki) m -> ki ko m", ki=P))
                nc.sync.dma_start(wk_s[:], wk.rearrange("(ko ki) m -> ki ko m", ki=P))
                nc.sync.dma_start(wv_s[:], wv.rearrange("(ko ki) m -> ki ko m", ki=P))

                for b in range(BPC):
                    sx = xin.tile([P, KS, N], bf16, tag="xin")
                    sq = xin.tile([P, KS, N], bf16, tag="xin")
                    nc.sync.dma_start(sx[:], xT[b].rearrange("(ko ki) n -> ki ko n", ki=P))
                    nc.sync.dma_start(sq[:], qT[b].rearrange("(ko ki) n -> ki ko n", ki=P))

                    # ones block of V_aug (odd 64-col halves of each 128 block)
                    va_v = VA[b].rearrange("p k (h c) -> p k h c", c=128)
                    nc.any.memset(va_v[:, :, :, HD:], 1.0)

                    # Q^T and K^T tiles: out[128 outdim, 740] = W^T @ act
                    for t in range(KS):
                        for w_s, src, dst, scl in ((wq_s, sq, QT[b], SCALE),
                                                   (wk_s, sx, KTT[b], 1.0)):
                            ps = psum.tile([P, DIM], f32, tag="ps")
                            for ks in range(KS):
                                for n0, n1 in NSPLITS:
                                    nc.tensor.matmul(
                                        ps[:, n0:n1],
                                        w_s[:, ks, t * P:(t + 1) * P],
                                        src[:, ks, n0:n1],
                                        start=(ks == 0), stop=(ks == KS - 1))
                            if scl != 1.0:
                                nc.vector.tensor_scalar_mul(dst[:, t, :], ps[:, :N], scl)
                            else:
                                nc.vector.tensor_copy(dst[:, t, :], ps[:, :N])

                    # V token-major: out[128 tok, 768] = x @ W_v
                    for i in range(KS):
                        nk = KT_SIZES[i]
                        ps = psum.tile([P, DIM], f32, tag="ps")
                        for ks in range(KS):
                            for d0, d1 in D_SPLITS:
                                nc.tensor.matmul(
                                    ps[:nk, d0:d1],
                                    sx[:, ks, i * P:i * P + nk],
                                    wv_s[:, ks, d0:d1],
                                    start=(ks == 0), stop=(ks == KS - 1))
                        nc.vector.tensor_copy(
                            va_v[:nk, i, :, :HD],
                            ps[:nk, :].rearrange("p (h c) -> p h c", h=HEADS))

            # ---------------- attention phase ----------------
            with tc.tile_pool(name="ebp", bufs=2) as ebp, \
                 tc.tile_pool(name="attw", bufs=4) as attw, \
                 tc.tile_pool(name="otp", bufs=1) as otp:
                OT = [otp.tile([P, KS, N], bf16, tag=f"ot{b}", name=f"ot{b}") for b in range(BPC)]
                for h in range(HEADS):
                    tq, off = h // 2, (h % 2) * HD
                    ebt = ebp.tile([P, KS, N], bf16, tag="eb")
                    nc.sync.dma_start(
                        ebt[:], eb[h].rearrange("(ko ki) n -> ki ko n", ki=P))
                    mule = nc.vector if h % 2 == 0 else nc.gpsimd
                    for b in range(BPC):
                        ops = psum.tile([P, DIM], f32, tag="ps")
                        for kt in range(KS):
                            nk = KT_SIZES[kt]
                            sps = psum.tile([P, DIM], f32, tag="ps")
                            pexp = attw.tile([P, N], bf16, tag="pexp")
                            for n0, n1 in NSPLITS:
                                nc.tensor.matmul(
                                    sps[:nk, n0:n1],
                                    KTT[b][off:off + HD, tq, kt * P:kt * P + nk],
                                    QT[b][off:off + HD, tq, n0:n1],
                                    start=True, stop=True)
                            nc.scalar.activation(pexp[:nk, :], sps[:nk, :N], EXP)
                            mule.tensor_mul(pexp[:nk, :], pexp[:nk, :],
                                            ebt[:nk, kt, :])
                            for n0, n1 in NSPLITS:
                                nc.tensor.matmul(
                                    ops[:, n0:n1],
                                    VA[b][:nk, kt, h * 128:(h + 1) * 128],
                                    pexp[:nk, n0:n1],
                                    start=(kt == 0), stop=(kt == KS - 1))
                        rs = attw.tile([HD, N], f32, tag="rs")
                        nc.vector.reciprocal(rs[:], ops[HD:128, :N])
                        nc.vector.tensor_mul(OT[b][off:off + HD, tq, :],
                                             ops[:HD, :N], rs[:])

                # ---------------- projection phase ----------------
                with tc.tile_pool(name="wpp", bufs=1) as wpp, \
                     tc.tile_pool(name="outp", bufs=3) as outp:
                    wp_s = wpp.tile([P, KS, DIM], bf16, tag="wp")
                    pb_s = wpp.tile([P, DIM], f32, tag="pb")
                    nc.sync.dma_start(wp_s[:], wp.rearrange("(ko ki) m -> ki ko m", ki=P))
                    nc.sync.dma_start(pb_s[:], pbr[:, :])
                    for b in range(BPC):
                        for i in range(KS):
                            nk = KT_SIZES[i]
                            ps = psum.tile([P, DIM], f32, tag="ps")
                            for ks in range(KS):
                                for d0, d1 in D_SPLITS:
                                    nc.tensor.matmul(
                                        ps[:nk, d0:d1],
                                        OT[b][:, ks, i * P:i * P + nk],
                                        wp_s[:, ks, d0:d1],
                                        start=(ks == 0), stop=(ks == KS - 1))
                            ob = outp.tile([P, DIM], f32, tag="ob")
                            nc.vector.tensor_add(ob[:nk, :], ps[:nk, :],
                                                 pb_s[:nk, :])
                            nc.sync.dma_start(out[b, i * P:i * P + nk, :],
                                              ob[:nk, :])
    nc.finalize()
    return nc


_NC_CACHE = None
LAST_RESULT = None


def kernel(**inputs):
    global _NC_CACHE
    from concourse.bass_utils import run_bass_kernel_spmd

    f = lambda k: np.asarray(inputs[k], np.float32)
    x, temp_q, target_q = f("x"), f("temp_q"), f("target_q")
    q_in = np.concatenate([temp_q, target_q], axis=1)            # [B,N,D]

    ebias = _host_bias(f("rpb_table"), f("rpb_table_temp"), f("pos_w1"),
                       f("pos_b1"), f("pos_w2"), f("pos_b2"),
                       inputs["target_temp_tab"])                # [H,N,N] fp32
    ebp = np.zeros((HEADS, DIM, N), BF16)
    ebp[:, :N, :] = ebias.astype(BF16)

    xT = np.ascontiguousarray(x.transpose(0, 2, 1)).astype(BF16)      # [B,D,N]
    qT = np.ascontiguousarray(q_in.transpose(0, 2, 1)).astype(BF16)
    shared = {
        "wq": f("q_w").astype(BF16), "wk": f("k_w").astype(BF16),
        "wv": f("v_w").astype(BF16), "wp": f("proj_w").astype(BF16),
        "pbr": np.broadcast_to(f("proj_b"), (P, DIM)).copy(),
        "eb": ebp,
    }
    in_maps = []
    for c in range(NCORES):
        m = dict(shared)
        m["xT"] = np.ascontiguousarray(xT[c * BPC:(c + 1) * BPC])
        m["qT"] = np.ascontiguousarray(qT[c * BPC:(c + 1) * BPC])
        in_maps.append(m)

    if _NC_CACHE is None:
        _NC_CACHE = _build_nc()
    res = run_bass_kernel_spmd(_NC_CACHE, in_maps, core_ids=list(range(NCORES)),
                               tmpdir=globals().get('PROF_TMPDIR'))
    global LAST_RESULT
    LAST_RESULT = res
    outs = [r["out"] for r in res.results]
    return np.concatenate(outs, axis=0)


if __name__ == "__main__":
    nc = _build_nc()
    print("built ok")
